# revision 48
# baseline (speedup 1.0000x reference)
"""GQA attention kernel for Trainium2, 8-core SPMD.

Sharding: tensor-parallel=4 over kv-head pairs x data-parallel=2 over batch.
Each core: one batch, 8 q-heads, 2 kv-heads, full 2048-token sequence.
All matmuls bf16 inputs / fp32 PSUM accumulation.

v3 design notes (v2 trace-driven):
  - attention inner loop: scores+exp emitted one k-tile AHEAD of the attn@V
    consumer, so the s2->exp->attn2 chain never serializes on PE's strict
    FIFO (v2 paid ~1.2us/iter of PE idle + HAM re-throttle for this)
  - block-3 Q projection is NOT done in phase 1: chunk-0 attention only needs
    block-0 Q, so block-3's Q runs as PE fill inside chunk-0's ACT(exp)-bound
    window (same trick as O-proj filling chunks 1-3)
  - startup: first matmul needs wk[0] chunk 0 + xt block-0 chunk 0; those DMAs
    interleave first (first MM at ~5us instead of ~19us)
  - softmax denominator: e_sum accumulated on DVE, one ones-matmul per head
    per q-chunk broadcasts the k-sum to all partitions
  - O-projection emitted as fine-grained 4-matmul chunks injected between
    attention k-tiles (PE never idles in bursts, HAM stays warm)
  - wq/wk host layout [h,p,c,m] so weight-slab DMAs are 8KB-contiguous
"""
import numpy as np
import ml_dtypes

import concourse.bacc as bacc
import concourse.bass as bass
import concourse.tile as tile
from concourse import mybir
from concourse.bass_utils import run_bass_kernel_spmd

BF = mybir.dt.bfloat16
F32 = mybir.dt.float32
BF_NP = np.dtype(ml_dtypes.bfloat16)

B, S, HIDDEN = 2, 2048, 4096
NUM_HEADS, NUM_KV_HEADS, HEAD_DIM = 32, 8, 128
GROUPS = NUM_HEADS // NUM_KV_HEADS
ROPE_THETA = 10000.0
TP = 4

FULL_CFG = dict(S=2048, HID=4096, NQ=8, NKV=2, SB=512, QC=512)


def build_nc(cfg):
    S_, HID, NQ, NKV, SB, QC = (cfg[k] for k in ("S", "HID", "NQ", "NKV", "SB", "QC"))
    HC = HID // 128
    NB = S_ // SB
    NQC = S_ // QC
    KT = S_ // 128
    DV = NKV * 128
    NO = HID // 512
    NHP = NQ // 2            # head pairs
    scale = 1.0 / np.sqrt(128.0)

    nc = bacc.Bacc("TRN2", target_bir_lowering=False, debug=False)
    xt = nc.dram_tensor("xt", (HC, 128, S_), BF, kind="ExternalInput").ap()
    wq = nc.dram_tensor("wq", (NQ, 128, HC, 128), BF, kind="ExternalInput").ap()
    wk = nc.dram_tensor("wk", (NKV, 128, HC, 128), BF, kind="ExternalInput").ap()
    wv = nc.dram_tensor("wv", (HC, 128, DV), BF, kind="ExternalInput").ap()
    wo = nc.dram_tensor("wo", (NO, 128, NQ, 512), BF, kind="ExternalInput").ap()
    cosd = nc.dram_tensor("cos", (128, S_), BF, kind="ExternalInput").ap()
    sind = nc.dram_tensor("sin", (128, S_), BF, kind="ExternalInput").ap()
    rmatd = nc.dram_tensor("rmat", (128, 128), BF, kind="ExternalInput").ap()
    identd = nc.dram_tensor("ident", (128, 128), BF, kind="ExternalInput").ap()
    o = nc.dram_tensor("o", (S_, HID), BF, kind="ExternalOutput").ap()

    with tile.TileContext(nc) as tc:
        with tc.tile_pool(name="cons", bufs=1) as cons, \
             tc.tile_pool(name="big", bufs=1) as big, \
             tc.tile_pool(name="csn", bufs=1) as csn, \
             tc.tile_pool(name="xt3p", bufs=1) as xt3p:
            r_sb = cons.tile([128, 128], BF, name="r_sb")
            id_sb = cons.tile([128, 128], BF, name="id_sb")
            ones_sb = cons.tile([128, 128], BF, name="ones_sb")
            nc.sync.dma_start(out=r_sb, in_=rmatd)
            nc.sync.dma_start(out=id_sb, in_=identd)
            nc.vector.memset(ones_sb, 1.0)

            q_sb = big.tile([128, NQ, S_], BF, name="q_sb")
            k_sb = big.tile([128, NKV, S_], BF, name="k_sb")
            v_sb = big.tile([128, KT, DV], BF, name="v_sb")
            cos_sb = csn.tile([128, S_], BF, name="cos_sb")
            sin_sb = csn.tile([128, S_], BF, name="sin_sb")
            xt3 = xt3p.tile([128, HC, SB], BF, name="xt3")

            xt_r = xt.rearrange("c p s -> p c s")

            # ---- phase 1a: K+V all blocks + Q blocks 0..NB-2, with rope ----
            with tc.tile_pool(name="wvp", bufs=1) as wvp, \
                 tc.tile_pool(name="xp", bufs=2) as xp, \
                 tc.tile_pool(name="wp", bufs=3) as wp, \
                 tc.tile_pool(name="rt", bufs=4) as rt, \
                 tc.tile_pool(name="vt", bufs=2) as vt, \
                 tc.tile_pool(name="pp", bufs=2, space="PSUM") as pp, \
                 tc.tile_pool(name="rp", bufs=2, space="PSUM") as rp, \
                 tc.tile_pool(name="vtp", bufs=2, space="PSUM") as vtp, \
                 tc.tile_pool(name="tpp", bufs=2, space="PSUM") as tpp:
                wv_sb = wvp.tile([128, HC, DV], BF, name="wv_sb")
                # startup-critical order: the first matmul needs wk[0] chunk 0
                # and xt block-0 chunk 0 -- interleave those DMAs first, then
                # the rest (cos/sin/wv follow; they're needed ~15us in)
                wk_first = [wp.tile([128, HC, 128], BF, name="w_slab")
                            for _ in range(NKV)]
                xt0 = xp.tile([128, HC, SB], BF, name="xt_t")
                # each dma_start costs the sync sequencer ~615ns to trigger, so
                # batch chunks: 4-wide only where the first matmuls need
                # progressive availability, 16-wide everywhere else
                for c4 in range(0, HC, 4):
                    nc.sync.dma_start(out=wk_first[0][:, c4:c4 + 4, :],
                                      in_=wk[0][:, c4:c4 + 4, :])
                    if c4 == 0:
                        # singles up front: the first matmul needs only
                        # wk0[0]+xt0[0], not a whole 4-group
                        for c in range(4):
                            nc.sync.dma_start(out=xt0[:, c, :],
                                              in_=xt_r[:, c, 0:SB])
                    else:
                        nc.sync.dma_start(out=xt0[:, c4:c4 + 4, :],
                                          in_=xt_r[:, c4:c4 + 4, 0:SB])
                for c16 in range(0, HC, 16):
                    nc.sync.dma_start(out=wk_first[1][:, c16:c16 + 16, :],
                                      in_=wk[1][:, c16:c16 + 16, :])
                nc.sync.dma_start(out=cos_sb, in_=cosd)
                nc.sync.dma_start(out=sin_sb, in_=sind)

                # delayed emission of PE ops whose inputs come off ACT/DVE,
                # so the PE queue never waits on a slow-engine producer
                pending = []

                def flush_pending():
                    while pending:
                        pending.pop(0)()

                wv_r = wv.rearrange("c p v -> p c v")
                for sb_i in range(NB):
                    ssl = slice(sb_i * SB, (sb_i + 1) * SB)
                    if sb_i == 0:
                        xt_t = xt0
                        for c16 in range(0, HC, 16):
                            nc.sync.dma_start(out=wv_sb[:, c16:c16 + 16, :],
                                              in_=wv_r[:, c16:c16 + 16, :])
                    else:
                        xt_t = xt3 if sb_i == NB - 1 else \
                            xp.tile([128, HC, SB], BF, name="xt_t")
                        for c16 in range(0, HC, 16):
                            nc.sync.dma_start(out=xt_t[:, c16:c16 + 16, :],
                                              in_=xt_r[:, c16:c16 + 16, ssl])

                    # weight-slab prefetch: slab h+1's DMA issues while h runs
                    slabq = []

                    def slab_for(wten, h, sb_i=sb_i):
                        if sb_i == 0 and wten is wk:
                            return wk_first[h]
                        t = wp.tile([128, HC, 128], BF, name="w_slab")
                        nc.sync.dma_start(out=t, in_=wten[h])
                        return t

                    # K projection + rope, then v^T + transpose, then Q + rope
                    # (block NB-1's Q is deferred into chunk-0 attention fill)
                    nq_here = 0 if sb_i == NB - 1 else NQ
                    plan = [("k", NKV, wk, k_sb), ("v", NKV, wv, None),
                            ("q", nq_here, wq, q_sb)]
                    slab_src = [(wten, h) for which, nheads, wten, _ in plan
                                if which != "v" for h in range(nheads)]
                    slabq = [slab_for(*s) for s in slab_src[:2]]
                    slab_i = 0
                    for which, nheads, wten, dst in plan:
                        for h in range(nheads):
                            if which == "v":
                                # v^T[d, tok] via long streams, then transpose
                                vt_ps = vtp.tile([128, SB], F32, name="vt_ps")
                                for c in range(HC):
                                    nc.tensor.matmul(
                                        vt_ps, wv_sb[:, c, h * 128:(h + 1) * 128],
                                        xt_t[:, c, :],
                                        start=(c == 0), stop=(c == HC - 1))
                                vt_sb = vt.tile([128, SB], BF, name="vt_sb")
                                nc.scalar.activation(
                                    out=vt_sb, in_=vt_ps,
                                    func=mybir.ActivationFunctionType.Copy)

                                def mk_trans(vt_sb=vt_sb, sb_i=sb_i, h=h):
                                    for t in range(SB // 128):
                                        tp_ps = tpp.tile([128, 128], BF,
                                                         name="tp_ps")
                                        nc.tensor.transpose(
                                            tp_ps,
                                            vt_sb[:, t * 128:(t + 1) * 128],
                                            id_sb)
                                        kt_i = sb_i * (SB // 128) + t
                                        nc.vector.tensor_copy(
                                            v_sb[:, kt_i, h * 128:(h + 1) * 128],
                                            tp_ps)
                                pending.append(mk_trans)
                                if len(pending) >= 2:
                                    pending.pop(0)()
                                continue
                            ps = pp.tile([128, SB], F32, name="ps_proj")
                            wslab = slabq[slab_i]
                            slab_i += 1
                            if slab_i + 1 < len(slab_src):
                                slabq.append(slab_for(*slab_src[slab_i + 1]))
                            for c in range(HC):
                                nc.tensor.matmul(ps, wslab[:, c, :], xt_t[:, c, :],
                                                 start=(c == 0), stop=(c == HC - 1))
                            # rope: dst = ps*cos + (R@ps)*sin_signed
                            qbf = rt.tile([128, SB], BF, name="rope_bf")
                            nc.scalar.activation(
                                out=qbf, in_=ps,
                                func=mybir.ActivationFunctionType.Copy)

                            def mk_rope(ps=ps, qbf=qbf, dst=dst, h=h, ssl=ssl):
                                rot = rp.tile([128, SB], F32, name="rot_ps")
                                nc.tensor.matmul(rot, r_sb, qbf,
                                                 start=True, stop=True)
                                t1 = rt.tile([128, SB], BF, name="rope_t1")
                                t2 = rt.tile([128, SB], BF, name="rope_t2")
                                nc.vector.tensor_mul(t1, ps, cos_sb[:, ssl])
                                nc.vector.tensor_mul(t2, rot, sin_sb[:, ssl])
                                nc.vector.tensor_add(dst[:, h, ssl], t1, t2)
                            pending.append(mk_rope)
                            if len(pending) >= 2:
                                pending.pop(0)()
                flush_pending()

            # ------------- phase 2+3: attention + output projection -------------
            with tc.tile_pool(name="aq", bufs=2) as aq, \
                 tc.tile_pool(name="ep", bufs=6) as ep, \
                 tc.tile_pool(name="es", bufs=3) as es, \
                 tc.tile_pool(name="rb", bufs=2) as rb, \
                 tc.tile_pool(name="ob", bufs=6) as ob, \
                 tc.tile_pool(name="sp", bufs=2, space="PSUM") as sp, \
                 tc.tile_pool(name="ap_", bufs=1, space="PSUM") as ap_:

                at_tiles = {}
                norm_pending = []

                def attention_chunk(qc, fill):
                    qsl = slice(qc * QC, (qc + 1) * QC)
                    at_qc = aq.tile([128, NQ, QC], BF, name="at_qc")
                    at_tiles[qc] = at_qc
                    for hp in range(NHP):
                        if norm_pending:
                            norm_pending.pop(0)()
                        h0 = 2 * hp
                        kvh = h0 // (NQ // NKV)
                        attn2 = ap_.tile([128, 2, QC], F32, name="attn2")
                        esd = es.tile([128, 2, QC], BF, name="esd")

                        # scores+exp emitted one k-tile AHEAD of attn2 so the
                        # s2->exp->attn2 chain never serializes on PE's FIFO
                        e2_tiles = {}

                        def emit_scores(kc):
                            s2 = sp.tile([128, 2, QC], F32, name="s2")
                            ksl = k_sb[:, kvh, kc * 128:(kc + 1) * 128]
                            nc.tensor.matmul(s2[:, 0, :], ksl, q_sb[:, h0, qsl],
                                             start=True, stop=True,
                                             skip_group_check=True)
                            nc.tensor.matmul(s2[:, 1, :], ksl,
                                             q_sb[:, h0 + 1, qsl],
                                             start=True, stop=True,
                                             skip_group_check=True)
                            e2 = esd if kc == 0 else ep.tile([128, 2, QC], BF,
                                                             name="e2")
                            nc.scalar.activation(
                                out=e2, in_=s2,
                                func=mybir.ActivationFunctionType.Exp,
                                scale=scale)
                            e2_tiles[kc] = e2

                        emit_scores(0)
                        for kc in range(KT):
                            if kc + 1 < KT:
                                emit_scores(kc + 1)
                            e2 = e2_tiles.pop(kc)
                            vsl = v_sb[:, kc, kvh * 128:(kvh + 1) * 128]
                            nc.tensor.matmul(attn2[:, 0, :], vsl, e2[:, 0, :],
                                             start=(kc == 0), stop=(kc == KT - 1),
                                             skip_group_check=True)
                            nc.tensor.matmul(attn2[:, 1, :], vsl, e2[:, 1, :],
                                             start=(kc == 0), stop=(kc == KT - 1),
                                             skip_group_check=True)
                            if kc > 0:
                                nc.vector.tensor_add(esd, esd, e2)
                            fill()
                        # den waits on the final esd DVE add -> give PE fill
                        fill()
                        fill()
                        # evict raw attention on ACT: once per 16 iterations
                        # ACT has slack, while DVE's pair-boundary burst
                        # (recip+mul) is what stalls the next chunk's oproj
                        at_un = es.tile([128, 2, QC], BF, name="at_un")
                        nc.scalar.activation(
                            out=at_un, in_=attn2,
                            func=mybir.ActivationFunctionType.Copy)
                        # den via all-ones stationary: every output partition
                        # gets the full k-sum, so the reciprocal is already
                        # broadcast. Rides the s2 slot ring to stay in 8 banks.
                        den_bc = sp.tile([128, 2, QC], F32, name="s2")
                        nc.tensor.matmul(den_bc[:, 0, :], ones_sb, esd[:, 0, :],
                                         start=True, stop=True,
                                         skip_group_check=True)
                        nc.tensor.matmul(den_bc[:, 1, :], ones_sb, esd[:, 1, :],
                                         start=True, stop=True,
                                         skip_group_check=True)
                        rec2 = rb.tile([128, 2, QC], F32, name="rec2")
                        nc.vector.reciprocal_approx_fast(out=rec2, in_=den_bc)
                        norm_pending.append(
                            lambda at_qc=at_qc, h0=h0, at_un=at_un, rec2=rec2:
                            nc.vector.tensor_mul(at_qc[:, h0:h0 + 2, :], at_un,
                                                 rec2))

                # ---- chunk 0: attention filled with block-3 Q projection ----
                with tc.tile_pool(name="wp2", bufs=3) as wp2, \
                     tc.tile_pool(name="rt2", bufs=2) as rt2, \
                     tc.tile_pool(name="pp1", bufs=1, space="PSUM") as pp1, \
                     tc.tile_pool(name="rp1", bufs=1, space="PSUM") as rp1:
                    ssl3 = slice((NB - 1) * SB, NB * SB)
                    qfill = []
                    ps3_live = {}
                    wslab3 = {}

                    def q3_slab(h):
                        t = wp2.tile([128, HC, 128], BF, name="w3_slab")
                        nc.sync.dma_start(out=t, in_=wq[h])
                        return t

                    def mk_proj_packet(h, g):
                        def f():
                            if g == 0:
                                ps3_live[h] = pp1.tile([128, SB], F32,
                                                       name="ps3")
                                if h + 2 < NQ:
                                    wslab3[h + 2] = q3_slab(h + 2)
                            ps = ps3_live[h]
                            wsl = wslab3[h]
                            for c in range(4 * g, 4 * g + 4):
                                nc.tensor.matmul(ps, wsl[:, c, :], xt3[:, c, :],
                                                 start=(c == 0),
                                                 stop=(c == HC - 1),
                                                 skip_group_check=True)
                        return f

                    def mk_rope_a(h):
                        def f():
                            # qbf eviction on DVE: ACT is chunk-0's pacer
                            qbf = rt2.tile([128, SB], BF, name="q3_bf")
                            nc.vector.tensor_copy(qbf, ps3_live[h])
                            ps3_live[(h, "bf")] = qbf
                        return f

                    def mk_rope_b(h):
                        def f():
                            ps = ps3_live.pop(h)
                            qbf = ps3_live.pop((h, "bf"))
                            del wslab3[h]
                            rot = rp1.tile([128, SB], F32, name="rot3")
                            nc.tensor.matmul(rot, r_sb, qbf,
                                             start=True, stop=True)
                            t1 = rt2.tile([128, SB], BF, name="q3_t1")
                            t2 = rt2.tile([128, SB], BF, name="q3_t2")
                            nc.vector.tensor_mul(t1, ps, cos_sb[:, ssl3])
                            nc.vector.tensor_mul(t2, rot, sin_sb[:, ssl3])
                            nc.vector.tensor_add(q_sb[:, h, ssl3], t1, t2)
                        return f

                    wslab3[0] = q3_slab(0)
                    wslab3[1] = q3_slab(1)
                    for h in range(NQ):
                        for g in range(HC // 4):
                            qfill.append(mk_proj_packet(h, g))
                        qfill.append(mk_rope_a(h))
                        qfill.append(mk_rope_b(h))

                    attention_chunk(0, lambda: qfill.pop(0)() if qfill else None)
                    while qfill:
                        qfill.pop(0)()

                # ---- chunks 1..3 + O-projection ----
                with tc.tile_pool(name="wob", bufs=1) as wob, \
                     tc.tile_pool(name="op", bufs=1, space="PSUM") as op:
                    wo_sb = wob.tile([128, NO, NQ, 512], BF, name="wo_sb")
                    for n in range(NO):
                        nc.sync.dma_start(out=wo_sb[:, n], in_=wo[n])

                    # O-projection as fine-grained 4-matmul chunks: single-n
                    # groups of 8 accumulating matmuls, split in half,
                    # alternating between 2 PSUM tiles.
                    oproj_chunks = []
                    op_live = [None, None]

                    def emit_oproj_chunk():
                        qc, tt, n, half, g = oproj_chunks.pop(0)
                        at_qc = at_tiles[qc]
                        tsl = slice(tt * 128, (tt + 1) * 128)
                        name = "ps_o" + "ab"[g % 2]
                        if half == 0:
                            t = op.tile([128, 512], F32, name=name)
                            op_live[g % 2] = t
                            for c in range(4):
                                nc.tensor.matmul(
                                    t, at_qc[:, c, tsl], wo_sb[:, n, c, :],
                                    start=(c == 0), stop=False,
                                    skip_group_check=True)
                        else:
                            t = op_live[g % 2]
                            for c in range(4, NQ):
                                nc.tensor.matmul(
                                    t, at_qc[:, c, tsl], wo_sb[:, n, c, :],
                                    start=False, stop=(c == NQ - 1),
                                    skip_group_check=True)
                            # always DVE: keep ACT free to run exp ahead
                            o_t = ob.tile([128, 512], BF, name="o_t")
                            nc.vector.tensor_copy(o_t, t)
                            nc.sync.dma_start(
                                out=o[qc * QC + tt * 128:
                                      qc * QC + (tt + 1) * 128,
                                      n * 512:(n + 1) * 512],
                                in_=o_t)

                    def queue_oproj(qc):
                        for g in range(32):
                            n = g // 4      # n-major: early groups need only
                            tt = g % 4      # the first wo chunks
                            oproj_chunks.append((qc, tt, n, 0, g))
                            oproj_chunks.append((qc, tt, n, 1, g))

                    queue_oproj(0)
                    for qc in range(1, NQC):
                        attention_chunk(
                            qc,
                            lambda: emit_oproj_chunk() if oproj_chunks
                            else None)
                        queue_oproj(qc)
                    # first drained group reads heads 0-3 (normalized long
                    # ago) -- emit it before the norm flush so PE has work
                    # while the last pair's norm chain runs on DVE
                    emit_oproj_chunk()
                    while norm_pending:
                        norm_pending.pop(0)()
                    while oproj_chunks:
                        emit_oproj_chunk()
    nc.compile()
    return nc


def _rope_tables(position_ids_b, S_):
    """cos/sin tables in [d=128, s] layout, sin sign-folded for the half-swap."""
    pos = position_ids_b.astype(np.float32)
    inv_freq = (1.0 / (ROPE_THETA ** (np.arange(0, HEAD_DIM, 2, dtype=np.float32)
                                      / HEAD_DIM))).astype(np.float32)
    freqs = pos[:, None] * inv_freq[None, :]          # [s, 64]
    emb = np.concatenate([freqs, freqs], axis=1)      # [s, 128]
    cos = np.cos(emb).T.copy()                        # [128, s]
    sin = np.sin(emb).T.copy()
    sin[:64] *= -1.0                                  # sign-fold for swap rope
    return cos.astype(BF_NP), sin.astype(BF_NP)


def _prep_core_inputs(hidden_states, position_ids, Wq, Wk, Wv, Wo):
    rmat = np.zeros((128, 128), dtype=np.float32)
    for i in range(128):
        rmat[i, (i + 64) % 128] = 1.0
    rmat = rmat.astype(BF_NP)
    ident = np.eye(128, dtype=np.float32).astype(BF_NP)

    HC = HIDDEN // 128
    in_maps = []
    for t in range(TP):
        fq = slice(1024 * t, 1024 * (t + 1))
        fkv = slice(256 * t, 256 * (t + 1))
        # [h, p, c, m] layout: slab DMA reads 8KB contiguous per partition
        wq_t = np.ascontiguousarray(
            Wq[:, fq].reshape(HC, 128, 8, 128).transpose(2, 1, 0, 3)).astype(BF_NP)
        wk_t = np.ascontiguousarray(
            Wk[:, fkv].reshape(HC, 128, 2, 128).transpose(2, 1, 0, 3)).astype(BF_NP)
        wv_t = np.ascontiguousarray(Wv[:, fkv].reshape(HC, 128, 256)).astype(BF_NP)
        wo_t = np.ascontiguousarray(
            Wo[fq, :].reshape(8, 128, 8, 512).transpose(2, 1, 0, 3)).astype(BF_NP)
        for b in range(B):
            xt = np.ascontiguousarray(
                hidden_states[b].T.reshape(HC, 128, S)).astype(BF_NP)
            cos, sin = _rope_tables(position_ids[b], S)
            in_maps.append({"xt": xt, "wq": wq_t, "wk": wk_t, "wv": wv_t,
                            "wo": wo_t, "cos": cos, "sin": sin, "rmat": rmat,
                            "ident": ident})
    return in_maps


_NC_CACHE = {}


def kernel(hidden_states, position_ids, Wq, Wk, Wv, Wo):
    if "nc" not in _NC_CACHE:
        _NC_CACHE["nc"] = build_nc(FULL_CFG)
    nc = _NC_CACHE["nc"]
    in_maps = _prep_core_inputs(np.asarray(hidden_states), np.asarray(position_ids),
                                np.asarray(Wq), np.asarray(Wk),
                                np.asarray(Wv), np.asarray(Wo))
    res = run_bass_kernel_spmd(nc, in_maps, core_ids=list(range(8)))
    out = np.zeros((B, S, HIDDEN), dtype=np.float32)
    for t in range(TP):
        for b in range(B):
            out[b] += res.results[t * B + b]["o"].astype(np.float32)
    return out
